# revision 9
# baseline (speedup 1.0000x reference)
"""Multi-head attention (B=2, S=2048, D=768, H=12) on 8 Trainium2 cores.

Sharding: core c -> batch b = c//4, heads 3*(c%4) .. 3*(c%4)+2.
Each core computes its 3 heads' attention weights (causal-specialized:
only the lower-triangular 128-row blocks are computed/written) and a
partial output projection; host sums the 4 partials per batch.

Device kernel is causal-mask specialized; if the mask input is not the
standard causal mask, a numpy fallback reproduces the reference exactly.
"""

import math

import numpy as np

import concourse.bass as bass
import concourse.mybir as mybir
import concourse.tile as tile
from concourse import bacc
from concourse.bass_utils import run_bass_kernel_spmd

B, S, D, H = 2, 2048, 768, 12
DK = D // H          # 64
HPC = 3              # heads per core
NCORES = 8
QB = 128             # q-block rows
NQB = S // QB        # 16
SB = 512             # q superblock (pass B moving free dim)
NSB = S // SB        # 4
DCH = D // 128       # 6 contraction chunks for projections
FP32 = mybir.dt.float32
FP16 = mybir.dt.float16
AX = mybir.AxisListType.X
ALU = mybir.AluOpType
EXP = mybir.ActivationFunctionType.Exp

_CACHE = {}


def _build_nc():
    nc = bacc.Bacc()

    qT = nc.dram_tensor("qT", [D, S], FP16, kind="ExternalInput")
    kT = nc.dram_tensor("kT", [D, S], FP16, kind="ExternalInput")
    vT = nc.dram_tensor("vT", [D, S], FP16, kind="ExternalInput")
    wq = nc.dram_tensor("wq", [D, HPC * DK], FP16, kind="ExternalInput")
    wk = nc.dram_tensor("wk", [D, HPC * DK], FP16, kind="ExternalInput")
    wv = nc.dram_tensor("wv", [D, HPC * DK], FP16, kind="ExternalInput")
    wo = nc.dram_tensor("wo", [HPC * DK, D], FP16, kind="ExternalInput")
    dmask = nc.dram_tensor("dmask", [QB, QB], FP32, kind="ExternalInput")
    dmaskT = nc.dram_tensor("dmaskT", [QB, QB], FP32, kind="ExternalInput")
    dmaskT16 = nc.dram_tensor("dmaskT16", [QB, QB], FP16, kind="ExternalInput")

    attn3 = nc.dram_tensor("attn3", [HPC, S, S], FP32, kind="ExternalOutput")
    out_part = nc.dram_tensor("out_part", [S, D], FP32, kind="ExternalOutput")

    with tile.TileContext(nc) as tc:
        with tc.tile_pool(name="consts", bufs=1) as consts:
            wq_t = [consts.tile([128, HPC * DK], FP16, tag=f"wq{d}", name=f"wq{d}") for d in range(DCH)]
            wk_t = [consts.tile([128, HPC * DK], FP16, tag=f"wk{d}", name=f"wk{d}") for d in range(DCH)]
            wv_t = [consts.tile([128, HPC * DK], FP16, tag=f"wv{d}", name=f"wv{d}") for d in range(DCH)]
            for d in range(DCH):
                nc.sync.dma_start(out=wq_t[d], in_=wq[d * 128:(d + 1) * 128, :])
                nc.sync.dma_start(out=wk_t[d], in_=wk[d * 128:(d + 1) * 128, :])
                nc.sync.dma_start(out=wv_t[d], in_=wv[d * 128:(d + 1) * 128, :])
            wo_t = [consts.tile([DK, D], FP16, tag=f"wo{h}", name=f"wo{h}") for h in range(HPC)]
            for h in range(HPC):
                nc.sync.dma_start(out=wo_t[h], in_=wo[h * DK:(h + 1) * DK, :])
            dm_t = consts.tile([QB, QB], FP32, tag="dm", name="dm")
            dmT_t = consts.tile([QB, QB], FP32, tag="dmT", name="dmT")
            nc.sync.dma_start(out=dm_t, in_=dmask[:, :])
            nc.sync.dma_start(out=dmT_t, in_=dmaskT[:, :])
            dmT16_t = consts.tile([QB, QB], FP16, tag="dmT16", name="dmT16")
            nc.sync.dma_start(out=dmT16_t, in_=dmaskT16[:, :])

            # persistent projected tensors
            qh = [consts.tile([DK, S], FP16, tag=f"qh{h}", name=f"qh{h}") for h in range(HPC)]
            kh = [consts.tile([DK, S], FP16, tag=f"kh{h}", name=f"kh{h}") for h in range(HPC)]
            v3 = [consts.tile([128, HPC * DK], FP16, tag=f"v3{s}", name=f"v3{s}") for s in range(NQB)]

            # ---- projections ----
            with tc.tile_pool(name="xt", bufs=7) as xt, \
                 tc.tile_pool(name="pjp01", bufs=2, space="PSUM") as pjp01, \
                 tc.tile_pool(name="pjp2", bufs=2, space="PSUM") as pjp2, \
                 tc.tile_pool(name="pjpv", bufs=2, space="PSUM") as pjpv:
                for (xdram, wt, outh) in ((qT, wq_t, qh), (kT, wk_t, kh)):
                    xts = []
                    for d in range(DCH):
                        t = xt.tile([128, S], FP16, tag="xt", name="xt")
                        nc.sync.dma_start(out=t, in_=xdram[d * 128:(d + 1) * 128, :])
                        xts.append(t)
                    for c in range(NSB):
                        cs = slice(c * SB, (c + 1) * SB)
                        ps = pjp01.tile([128, SB], FP32, tag="pj01", name="pj01")
                        for d in range(DCH):
                            nc.tensor.matmul(ps, wt[d][:, 0:128], xts[d][:, cs],
                                             start=(d == 0), stop=(d == DCH - 1))
                        nc.vector.tensor_copy(out=outh[0][:, cs], in_=ps[0:DK, :])
                        nc.vector.tensor_copy(out=outh[1][:, cs], in_=ps[DK:128, :])
                        ps2 = pjp2.tile([DK, SB], FP32, tag="pj2", name="pj2")
                        for d in range(DCH):
                            nc.tensor.matmul(ps2, wt[d][:, 128:192], xts[d][:, cs],
                                             start=(d == 0), stop=(d == DCH - 1))
                        nc.vector.tensor_copy(out=outh[2][:, cs], in_=ps2)
                # value: natural layout, 3 heads side by side
                xts = []
                for d in range(DCH):
                    t = xt.tile([128, S], FP16, tag="xt", name="xt")
                    nc.sync.dma_start(out=t, in_=vT[d * 128:(d + 1) * 128, :])
                    xts.append(t)
                for s in range(NQB):
                    ss = slice(s * 128, (s + 1) * 128)
                    psv = pjpv.tile([128, HPC * DK], FP32, tag="pjv", name="pjv")
                    for d in range(DCH):
                        nc.tensor.matmul(psv, xts[d][:, ss], wv_t[d],
                                         start=(d == 0), stop=(d == DCH - 1))
                    nc.vector.tensor_copy(out=v3[s], in_=psv)

            # ---- attention ----
            with tc.tile_pool(name="scps", bufs=4, space="PSUM") as scps, \
                 tc.tile_pool(name="ctxps", bufs=2, space="PSUM") as ctxps, \
                 tc.tile_pool(name="wops", bufs=1, space="PSUM") as wops, \
                 tc.tile_pool(name="rowp", bufs=4) as rowp, \
                 tc.tile_pool(name="expp", bufs=6) as expp, \
                 tc.tile_pool(name="ctxsb", bufs=2) as ctxsbp, \
                 tc.tile_pool(name="outacc", bufs=2) as outaccp, \
                 tc.tile_pool(name="statp", bufs=8) as statp, \
                 tc.tile_pool(name="recp", bufs=2) as recp:
                for sb in range(NSB):
                    recips = recp.tile([128, HPC * 4], FP32, tag="recips", name="recips")
                    # ---------- pass A: softmax rows + attn output ----------
                    for h in range(HPC):
                        for qi in range(4):
                            i = sb * 4 + qi
                            L = (i + 1) * QB
                            nch = math.ceil(L / SB)
                            col = h * 4 + qi
                            row = rowp.tile([128, S], FP32, tag="row", name="row")
                            sums = statp.tile([128, 4], FP32, tag="sums", name="sums")
                            for c in range(nch):
                                w = min(SB, L - c * SB)
                                ps = scps.tile([128, SB], FP32, tag="sc", name="sc")
                                nc.tensor.matmul(
                                    ps[:, :w],
                                    qh[h][:, i * QB:(i + 1) * QB],
                                    kh[h][:, c * SB:c * SB + w],
                                    start=True, stop=True)
                                if c < nch - 1:
                                    nc.scalar.activation(
                                        out=row[:, c * SB:c * SB + w], in_=ps[:, :w],
                                        func=EXP, accum_out=sums[:, c:c + 1])
                                else:
                                    nc.scalar.activation(
                                        out=row[:, c * SB:c * SB + w], in_=ps[:, :w],
                                        func=EXP)
                                    nc.vector.tensor_mul(
                                        row[:, L - QB:L], row[:, L - QB:L], dm_t)
                                    nc.vector.reduce_sum(
                                        out=sums[:, c:c + 1],
                                        in_=row[:, c * SB:c * SB + w], axis=AX)
                            tot = statp.tile([128, 1], FP32, tag="tot", name="tot")
                            nc.vector.reduce_sum(out=tot, in_=sums[:, :nch], axis=AX)
                            nc.vector.reciprocal(
                                out=recips[:, col:col + 1], in_=tot)
                            nc.vector.tensor_scalar_mul(
                                row[:, :L], row[:, :L], recips[:, col:col + 1])
                            nc.gpsimd.dma_start(
                                out=attn3[h, i * QB:(i + 1) * QB, 0:L],
                                in_=row[:, :L])
                    # ---------- pass B: context + output projection ----------
                    outacc = [None] * 4
                    for h in range(HPC):
                        ctx = ctxps.tile([DK, SB], FP32, tag="ctx", name="ctx")
                        nj = sb * 4 + 4
                        for j in range(nj):
                            off = max(0, j * QB - sb * SB)
                            wq_cols = SB - off
                            ps = scps.tile([128, SB], FP32, tag="sc", name="sc")
                            nc.tensor.matmul(
                                ps[:, :wq_cols],
                                kh[h][:, j * QB:(j + 1) * QB],
                                qh[h][:, sb * SB + off:(sb + 1) * SB],
                                start=True, stop=True)
                            et = expp.tile([128, SB], FP16, tag="expT", name="expT")
                            nc.scalar.activation(
                                out=et[:, :wq_cols], in_=ps[:, :wq_cols], func=EXP)
                            if j >= sb * 4:
                                nc.vector.tensor_mul(
                                    et[:, 0:QB], et[:, 0:QB], dmT16_t)
                            nc.tensor.matmul(
                                ctx[:, off:SB],
                                v3[j][:, h * DK:(h + 1) * DK],
                                et[:, :wq_cols],
                                start=(j == 0), stop=(j == nj - 1))
                        ctxsb = ctxsbp.tile([DK, SB], FP16, tag="ctxsb", name="ctxsb")
                        nc.scalar.copy(out=ctxsb, in_=ctx)
                        for qi in range(4):
                            col = h * 4 + qi
                            wo_ps = wops.tile([128, D], FP32, tag="wo", name="wo")
                            nc.tensor.matmul(
                                wo_ps[:, 0:512],
                                ctxsb[:, qi * QB:(qi + 1) * QB],
                                wo_t[h][:, 0:512], start=True, stop=True)
                            nc.tensor.matmul(
                                wo_ps[:, 512:D],
                                ctxsb[:, qi * QB:(qi + 1) * QB],
                                wo_t[h][:, 512:D], start=True, stop=True)
                            if h == 0:
                                acc = outaccp.tile([128, D], FP32, tag=f"oa{qi}", name=f"oa{qi}")
                                outacc[qi] = acc
                                nc.vector.tensor_scalar_mul(
                                    acc, wo_ps, recips[:, col:col + 1])
                            else:
                                nc.vector.scalar_tensor_tensor(
                                    out=outacc[qi], in0=wo_ps,
                                    scalar=recips[:, col:col + 1],
                                    in1=outacc[qi],
                                    op0=ALU.mult, op1=ALU.add)
                    for qi in range(4):
                        i = sb * 4 + qi
                        nc.sync.dma_start(
                            out=out_part[i * QB:(i + 1) * QB, :],
                            in_=outacc[qi])
    nc.finalize()
    return nc


def _is_causal(mask):
    m = np.asarray(mask)
    if m.shape != (1, 1, S, S):
        return False
    return np.array_equal(m[0, 0], np.triu(np.ones((S, S), dtype=bool), k=1))


def _fallback(query, key, value, mask, w_q, w_k, w_v, w_o):
    scale = 1.0 / math.sqrt(DK)
    out = np.empty((B, S, D), np.float32)
    attn = np.empty((B, H, S, S), np.float32)
    m = np.asarray(mask)[0, 0]
    for b in range(B):
        q = (query[b] @ w_q.T).reshape(S, H, DK).transpose(1, 0, 2)
        k = (key[b] @ w_k.T).reshape(S, H, DK).transpose(1, 0, 2)
        v = (value[b] @ w_v.T).reshape(S, H, DK).transpose(1, 0, 2)
        for h in range(H):
            s = (q[h] @ k[h].T) * scale
            s = np.where(m, np.float32(-1e9), s)
            s = s - s.max(axis=-1, keepdims=True)
            e = np.exp(s)
            a = e / e.sum(axis=-1, keepdims=True)
            attn[b, h] = a
            ctx = a @ v[h]
            if h == 0:
                ob = np.zeros((S, D), np.float32)
            ob += ctx @ w_o[:, h * DK:(h + 1) * DK].T
        out[b] = ob
    return out, attn


def kernel(query, key, value, mask, w_q, w_k, w_v, w_o):
    query = np.ascontiguousarray(np.asarray(query, np.float32))
    key = np.ascontiguousarray(np.asarray(key, np.float32))
    value = np.ascontiguousarray(np.asarray(value, np.float32))
    w_q = np.asarray(w_q, np.float32)
    w_k = np.asarray(w_k, np.float32)
    w_v = np.asarray(w_v, np.float32)
    w_o = np.asarray(w_o, np.float32)

    if not _is_causal(mask):
        return _fallback(query, key, value, mask, w_q, w_k, w_v, w_o)

    if "nc" not in _CACHE:
        _CACHE["nc"] = _build_nc()
    nc = _CACHE["nc"]

    scale = np.float32(1.0 / math.sqrt(DK))
    dm = np.tril(np.ones((QB, QB), np.float32))
    dmT = np.ascontiguousarray(dm.T)

    in_maps = []
    for c in range(NCORES):
        b = c // 4
        h0 = HPC * (c % 4)
        hs = slice(h0 * DK, (h0 + HPC) * DK)
        in_maps.append({
            "qT": np.ascontiguousarray(query[b].T.astype(np.float16)),
            "kT": np.ascontiguousarray(key[b].T.astype(np.float16)),
            "vT": np.ascontiguousarray(value[b].T.astype(np.float16)),
            "wq": np.ascontiguousarray((w_q[hs, :] * scale).T.astype(np.float16)),
            "wk": np.ascontiguousarray(w_k[hs, :].T.astype(np.float16)),
            "wv": np.ascontiguousarray(w_v[hs, :].T.astype(np.float16)),
            "wo": np.ascontiguousarray(w_o[:, hs].T.astype(np.float16)),
            "dmask": dm,
            "dmaskT": dmT,
            "dmaskT16": dmT.astype(np.float16).copy(),
        })

    res = run_bass_kernel_spmd(nc, in_maps, core_ids=list(range(NCORES)))
    results = res.results

    attn = np.zeros((B, H, S, S), np.float32)
    out = np.zeros((B, S, D), np.float32)
    for c in range(NCORES):
        b = c // 4
        h0 = HPC * (c % 4)
        a3 = results[c]["attn3"]
        for hr in range(HPC):
            for i in range(NQB):
                L = (i + 1) * QB
                attn[b, h0 + hr, i * QB:L, :L] = a3[hr, i * QB:L, :L]
        out[b] += results[c]["out_part"]
    return out, attn


# revision 13
# speedup vs baseline: 1.0977x; 1.0977x over previous
"""Multi-head attention (B=2, S=2048, D=768, H=12) on 8 Trainium2 cores.

Sharding: core c -> batch b = c//4, heads 3*(c%4) .. 3*(c%4)+2.
Each core computes its 3 heads' attention weights (causal-specialized:
only the lower-triangular 128-row blocks are computed/written) and a
partial output projection; host sums the 4 partials per batch.

Device kernel is causal-mask specialized; if the mask input is not the
standard causal mask, a numpy fallback reproduces the reference exactly.
"""

import math

import numpy as np

import concourse.bass as bass
import concourse.mybir as mybir
import concourse.tile as tile
from concourse import bacc
from concourse.bass_utils import run_bass_kernel_spmd

B, S, D, H = 2, 2048, 768, 12
DK = D // H          # 64
HPC = 3              # heads per core
NCORES = 8
QB = 128             # q-block rows
NQB = S // QB        # 16
SB = 512             # q superblock (pass B moving free dim)
NSB = S // SB        # 4
DCH = D // 128       # 6 contraction chunks for projections
FP32 = mybir.dt.float32
FP16 = mybir.dt.float16
AX = mybir.AxisListType.X
ALU = mybir.AluOpType
EXP = mybir.ActivationFunctionType.Exp

_CACHE = {}


def _build_nc():
    nc = bacc.Bacc()

    qT = nc.dram_tensor("qT", [D, S], FP16, kind="ExternalInput")
    kT = nc.dram_tensor("kT", [D, S], FP16, kind="ExternalInput")
    vT = nc.dram_tensor("vT", [D, S], FP16, kind="ExternalInput")
    wq = nc.dram_tensor("wq", [D, HPC * DK], FP16, kind="ExternalInput")
    wk = nc.dram_tensor("wk", [D, HPC * DK], FP16, kind="ExternalInput")
    wv = nc.dram_tensor("wv", [D, HPC * DK], FP16, kind="ExternalInput")
    wo = nc.dram_tensor("wo", [HPC * DK, D], FP16, kind="ExternalInput")
    dmask = nc.dram_tensor("dmask", [QB, QB], FP32, kind="ExternalInput")
    dmaskT = nc.dram_tensor("dmaskT", [QB, QB], FP32, kind="ExternalInput")
    dmaskT16 = nc.dram_tensor("dmaskT16", [QB, QB], FP16, kind="ExternalInput")

    attn3 = nc.dram_tensor("attn3", [HPC, S, S], FP32, kind="ExternalOutput")
    out_part = nc.dram_tensor("out_part", [S, D], FP32, kind="ExternalOutput")

    with tile.TileContext(nc) as tc:
        with tc.tile_pool(name="consts", bufs=1) as consts:
            wq_t = [consts.tile([128, HPC * DK], FP16, tag=f"wq{d}", name=f"wq{d}") for d in range(DCH)]
            wk_t = [consts.tile([128, HPC * DK], FP16, tag=f"wk{d}", name=f"wk{d}") for d in range(DCH)]
            wv_t = [consts.tile([128, HPC * DK], FP16, tag=f"wv{d}", name=f"wv{d}") for d in range(DCH)]
            for d in range(DCH):
                nc.sync.dma_start(out=wq_t[d], in_=wq[d * 128:(d + 1) * 128, :])
                nc.sync.dma_start(out=wk_t[d], in_=wk[d * 128:(d + 1) * 128, :])
                nc.sync.dma_start(out=wv_t[d], in_=wv[d * 128:(d + 1) * 128, :])
            wo_t = [consts.tile([DK, D], FP16, tag=f"wo{h}", name=f"wo{h}") for h in range(HPC)]
            for h in range(HPC):
                nc.sync.dma_start(out=wo_t[h], in_=wo[h * DK:(h + 1) * DK, :])
            dm_t = consts.tile([QB, QB], FP32, tag="dm", name="dm")
            dmT_t = consts.tile([QB, QB], FP32, tag="dmT", name="dmT")
            nc.sync.dma_start(out=dm_t, in_=dmask[:, :])
            nc.sync.dma_start(out=dmT_t, in_=dmaskT[:, :])
            dmT16_t = consts.tile([QB, QB], FP16, tag="dmT16", name="dmT16")
            nc.sync.dma_start(out=dmT16_t, in_=dmaskT16[:, :])

            # persistent projected tensors
            qh = [consts.tile([DK, S], FP16, tag=f"qh{h}", name=f"qh{h}") for h in range(HPC)]
            kh = [consts.tile([DK, S], FP16, tag=f"kh{h}", name=f"kh{h}") for h in range(HPC)]
            v3 = [consts.tile([128, HPC * DK], FP16, tag=f"v3{s}", name=f"v3{s}") for s in range(NQB)]

            # ---- projections ----
            with tc.tile_pool(name="xt", bufs=7) as xt, \
                 tc.tile_pool(name="pjp01", bufs=2, space="PSUM") as pjp01, \
                 tc.tile_pool(name="pjp2", bufs=2, space="PSUM") as pjp2, \
                 tc.tile_pool(name="pjpv", bufs=2, space="PSUM") as pjpv:
                for (xdram, wt, outh) in ((qT, wq_t, qh), (kT, wk_t, kh)):
                    xts = []
                    for d in range(DCH):
                        t = xt.tile([128, S], FP16, tag="xt", name="xt")
                        nc.sync.dma_start(out=t, in_=xdram[d * 128:(d + 1) * 128, :])
                        xts.append(t)
                    for c in range(NSB):
                        cs = slice(c * SB, (c + 1) * SB)
                        ps = pjp01.tile([128, SB], FP32, tag="pj01", name="pj01")
                        for d in range(DCH):
                            nc.tensor.matmul(ps, wt[d][:, 0:128], xts[d][:, cs],
                                             start=(d == 0), stop=(d == DCH - 1))
                        nc.vector.tensor_copy(out=outh[0][:, cs], in_=ps[0:DK, :])
                        nc.vector.tensor_copy(out=outh[1][:, cs], in_=ps[DK:128, :])
                        ps2 = pjp2.tile([DK, SB], FP32, tag="pj2", name="pj2")
                        for d in range(DCH):
                            nc.tensor.matmul(ps2, wt[d][:, 128:192], xts[d][:, cs],
                                             start=(d == 0), stop=(d == DCH - 1))
                        nc.vector.tensor_copy(out=outh[2][:, cs], in_=ps2)
                # value: natural layout, 3 heads side by side
                xts = []
                for d in range(DCH):
                    t = xt.tile([128, S], FP16, tag="xt", name="xt")
                    nc.sync.dma_start(out=t, in_=vT[d * 128:(d + 1) * 128, :])
                    xts.append(t)
                for s in range(NQB):
                    ss = slice(s * 128, (s + 1) * 128)
                    psv = pjpv.tile([128, HPC * DK], FP32, tag="pjv", name="pjv")
                    for d in range(DCH):
                        nc.tensor.matmul(psv, xts[d][:, ss], wv_t[d],
                                         start=(d == 0), stop=(d == DCH - 1))
                    nc.vector.tensor_copy(out=v3[s], in_=psv)

            # ---- attention ----
            SB2 = 1024
            with tc.tile_pool(name="scpsA", bufs=2, space="PSUM") as scpsA, \
                 tc.tile_pool(name="scps", bufs=3, space="PSUM") as scps, \
                 tc.tile_pool(name="ctxps", bufs=1, space="PSUM") as ctxps, \
                 tc.tile_pool(name="rowp", bufs=4) as rowp, \
                 tc.tile_pool(name="expp", bufs=6) as expp, \
                 tc.tile_pool(name="ctxsb", bufs=2) as ctxsbp, \
                 tc.tile_pool(name="outacc", bufs=2) as outaccp, \
                 tc.tile_pool(name="statp", bufs=8) as statp, \
                 tc.tile_pool(name="recp", bufs=2) as recp:
                for sb in range(NSB):
                    recips = recp.tile([128, HPC * 4], FP32, tag="recips", name="recips")
                    # ---------- pass A: softmax rows + attn output ----------
                    for h in range(HPC):
                        for qi in range(4):
                            i = sb * 4 + qi
                            L = (i + 1) * QB
                            nch = math.ceil(L / SB2)
                            col = h * 4 + qi
                            row = rowp.tile([128, S], FP32, tag="row", name="row")
                            sums = statp.tile([128, 4], FP32, tag="sums", name="sums")
                            for c in range(nch):
                                w = min(SB2, L - c * SB2)
                                ps = scpsA.tile([128, SB2], FP32, tag="scA", name="scA")
                                for m in range(math.ceil(w / SB)):
                                    wm = min(SB, w - m * SB)
                                    nc.tensor.matmul(
                                        ps[:, m * SB:m * SB + wm],
                                        qh[h][:, i * QB:(i + 1) * QB],
                                        kh[h][:, c * SB2 + m * SB:c * SB2 + m * SB + wm],
                                        start=True, stop=True)
                                if c < nch - 1:
                                    nc.scalar.activation(
                                        out=row[:, c * SB2:c * SB2 + w], in_=ps[:, :w],
                                        func=EXP, accum_out=sums[:, c:c + 1])
                                else:
                                    nc.scalar.activation(
                                        out=row[:, c * SB2:c * SB2 + w], in_=ps[:, :w],
                                        func=EXP)
                                    nc.vector.tensor_mul(
                                        row[:, L - QB:L], row[:, L - QB:L], dm_t)
                                    nc.vector.reduce_sum(
                                        out=sums[:, c:c + 1],
                                        in_=row[:, c * SB2:c * SB2 + w], axis=AX)
                            tot = statp.tile([128, 1], FP32, tag="tot", name="tot")
                            nc.vector.reduce_sum(out=tot, in_=sums[:, :nch], axis=AX)
                            nc.vector.reciprocal(
                                out=recips[:, col:col + 1], in_=tot)
                            nc.vector.tensor_scalar_mul(
                                row[:, :L], row[:, :L], recips[:, col:col + 1])
                            nc.gpsimd.dma_start(
                                out=attn3[h, i * QB:(i + 1) * QB, 0:L],
                                in_=row[:, :L])
                    # ---------- pass B: context + output projection ----------
                    outacc = [None] * 4
                    for h in range(HPC):
                        ctx = ctxps.tile([DK, SB], FP32, tag="ctx", name="ctx")
                        nj = sb * 4 + 4
                        for j in range(nj):
                            off = max(0, j * QB - sb * SB)
                            wq_cols = SB - off
                            ps = scps.tile([128, SB], FP32, tag="sc", name="sc")
                            nc.tensor.matmul(
                                ps[:, :wq_cols],
                                kh[h][:, j * QB:(j + 1) * QB],
                                qh[h][:, sb * SB + off:(sb + 1) * SB],
                                start=True, stop=True)
                            et = expp.tile([128, SB], FP16, tag="expT", name="expT")
                            nc.scalar.activation(
                                out=et[:, :wq_cols], in_=ps[:, :wq_cols], func=EXP)
                            if j >= sb * 4:
                                nc.vector.tensor_mul(
                                    et[:, 0:QB], et[:, 0:QB], dmT16_t)
                            nc.tensor.matmul(
                                ctx[:, off:SB],
                                v3[j][:, h * DK:(h + 1) * DK],
                                et[:, :wq_cols],
                                start=(j == 0), stop=(j == nj - 1))
                        ctxsb = ctxsbp.tile([DK, SB], FP16, tag="ctxsb", name="ctxsb")
                        nc.vector.tensor_copy(out=ctxsb, in_=ctx)
                        for qi in range(4):
                            col = h * 4 + qi
                            wo_ps = scpsA.tile([128, SB2], FP32, tag="scA", name="wops")
                            nc.tensor.matmul(
                                wo_ps[:, 0:512],
                                ctxsb[:, qi * QB:(qi + 1) * QB],
                                wo_t[h][:, 0:512], start=True, stop=True)
                            nc.tensor.matmul(
                                wo_ps[:, 512:D],
                                ctxsb[:, qi * QB:(qi + 1) * QB],
                                wo_t[h][:, 512:D], start=True, stop=True)
                            if h == 0:
                                acc = outaccp.tile([128, D], FP32, tag=f"oa{qi}", name=f"oa{qi}")
                                outacc[qi] = acc
                                nc.vector.tensor_scalar_mul(
                                    acc, wo_ps[:, :D], recips[:, col:col + 1])
                            else:
                                nc.vector.scalar_tensor_tensor(
                                    out=outacc[qi], in0=wo_ps[:, :D],
                                    scalar=recips[:, col:col + 1],
                                    in1=outacc[qi],
                                    op0=ALU.mult, op1=ALU.add)
                    for qi in range(4):
                        i = sb * 4 + qi
                        nc.sync.dma_start(
                            out=out_part[i * QB:(i + 1) * QB, :],
                            in_=outacc[qi])
    nc.finalize()
    return nc


def _is_causal(mask):
    m = np.asarray(mask)
    if m.shape != (1, 1, S, S):
        return False
    return np.array_equal(m[0, 0], np.triu(np.ones((S, S), dtype=bool), k=1))


def _fallback(query, key, value, mask, w_q, w_k, w_v, w_o):
    scale = 1.0 / math.sqrt(DK)
    out = np.empty((B, S, D), np.float32)
    attn = np.empty((B, H, S, S), np.float32)
    m = np.asarray(mask)[0, 0]
    for b in range(B):
        q = (query[b] @ w_q.T).reshape(S, H, DK).transpose(1, 0, 2)
        k = (key[b] @ w_k.T).reshape(S, H, DK).transpose(1, 0, 2)
        v = (value[b] @ w_v.T).reshape(S, H, DK).transpose(1, 0, 2)
        for h in range(H):
            s = (q[h] @ k[h].T) * scale
            s = np.where(m, np.float32(-1e9), s)
            s = s - s.max(axis=-1, keepdims=True)
            e = np.exp(s)
            a = e / e.sum(axis=-1, keepdims=True)
            attn[b, h] = a
            ctx = a @ v[h]
            if h == 0:
                ob = np.zeros((S, D), np.float32)
            ob += ctx @ w_o[:, h * DK:(h + 1) * DK].T
        out[b] = ob
    return out, attn


def kernel(query, key, value, mask, w_q, w_k, w_v, w_o):
    query = np.ascontiguousarray(np.asarray(query, np.float32))
    key = np.ascontiguousarray(np.asarray(key, np.float32))
    value = np.ascontiguousarray(np.asarray(value, np.float32))
    w_q = np.asarray(w_q, np.float32)
    w_k = np.asarray(w_k, np.float32)
    w_v = np.asarray(w_v, np.float32)
    w_o = np.asarray(w_o, np.float32)

    if not _is_causal(mask):
        return _fallback(query, key, value, mask, w_q, w_k, w_v, w_o)

    if "nc" not in _CACHE:
        _CACHE["nc"] = _build_nc()
    nc = _CACHE["nc"]

    scale = np.float32(1.0 / math.sqrt(DK))
    dm = np.tril(np.ones((QB, QB), np.float32))
    dmT = np.ascontiguousarray(dm.T)

    in_maps = []
    for c in range(NCORES):
        b = c // 4
        h0 = HPC * (c % 4)
        hs = slice(h0 * DK, (h0 + HPC) * DK)
        in_maps.append({
            "qT": np.ascontiguousarray(query[b].T.astype(np.float16)),
            "kT": np.ascontiguousarray(key[b].T.astype(np.float16)),
            "vT": np.ascontiguousarray(value[b].T.astype(np.float16)),
            "wq": np.ascontiguousarray((w_q[hs, :] * scale).T.astype(np.float16)),
            "wk": np.ascontiguousarray(w_k[hs, :].T.astype(np.float16)),
            "wv": np.ascontiguousarray(w_v[hs, :].T.astype(np.float16)),
            "wo": np.ascontiguousarray(w_o[:, hs].T.astype(np.float16)),
            "dmask": dm,
            "dmaskT": dmT,
            "dmaskT16": dmT.astype(np.float16).copy(),
        })

    res = run_bass_kernel_spmd(nc, in_maps, core_ids=list(range(NCORES)))
    results = res.results

    attn = np.zeros((B, H, S, S), np.float32)
    out = np.zeros((B, S, D), np.float32)
    for c in range(NCORES):
        b = c // 4
        h0 = HPC * (c % 4)
        a3 = results[c]["attn3"]
        for hr in range(HPC):
            for i in range(NQB):
                L = (i + 1) * QB
                attn[b, h0 + hr, i * QB:L, :L] = a3[hr, i * QB:L, :L]
        out[b] += results[c]["out_part"]
    return out, attn
